# revision 1
# baseline (speedup 1.0000x reference)
"""Trainium2 Bass kernel for nn_BMManager: Linear([B,S,1024]->[B,S,512]) + bias,
then per-row segment forward-fill (expand_goals).

Strategy (data-parallel over batch, 8 cores x 4 batch rows each):
  out[b,t] = y[b, idx(b,t)] where y = x @ W^T + b and idx is a per-row
  forward-fill index (cummax of segment starts). Since idx(t) <= t and the
  fill only *gathers* rows of y, we commute the gather with the matmul:
      out[r] = x[idx(r)] @ W^T + bias
  So the kernel: (1) computes idx on-chip via a native prefix-max scan,
  (2) gathers x rows with the SWDGE dma_gather engine op,
  (3) casts to bf16, PE-transposes x^T blocks, runs bf16 matmuls
      (fp32 PSUM accumulation; gemm="f32r" fallback keeps tf32-like fp32r),
  (4) adds bias on the vector engine and streams rows straight to the output.

Per core: R=16384 rows (4 batches x 4096), D=1024, G=512.
Measured on trn2: ~378-381 us HW exec (best single run 363 us), rel err
2.4e-3 (bf16); gemm="f32r" fallback: ~433 us, rel err 1.5e-4.
"""

import numpy as np

import concourse.bacc as bacc
import concourse.bass as bass
import concourse.mybir as mybir
import concourse.tile as tile
from concourse.bass_utils import run_bass_kernel_spmd
from concourse.masks import make_identity

P = 128
N_CORES = 8
B_FULL, S, D_IN, D_GOAL = 32, 4096, 1024, 512
B_PC = B_FULL // N_CORES          # 4 batch rows per core
R = B_PC * S                      # 16384 rows per core
NB = R // (P * 16)                # 8 "B-blocks" of 2048 rows in the scan layout
K_TILES = D_IN // P               # 8
N_GATHER = 512                    # rows per dma_gather call
N_G = R // N_GATHER               # 32 gather calls
CHUNKS_PER_G = N_GATHER // P      # 4
NQ = 4                            # swdge queues

F32 = mybir.dt.float32
F32R = mybir.dt.float32r
I32 = mybir.dt.int32
I16 = mybir.dt.int16
U8 = mybir.dt.uint8


def ts(i, n):
    return slice(i * n, (i + 1) * n)


BF16 = mybir.dt.bfloat16


def build_program(gemm="bf16"):
    """gemm: 'bf16' (cast x/W to bf16; fastest PE path) or 'f32r' (tf32-like)."""
    matmul_dtype = BF16 if gemm == "bf16" else F32R
    nc = bacc.Bacc(
        "TRN2",
        target_bir_lowering=False,
        debug=False,
        num_devices=N_CORES,
        num_swdge_queues=NQ,
        use_seq_codegen=True,
    )
    x_d = nc.dram_tensor("x", [R, D_IN], F32, kind="ExternalInput")
    msk_d = nc.dram_tensor("msk", [B_PC, S], U8, kind="ExternalInput")
    w_d = nc.dram_tensor("w", [D_GOAL, D_IN], F32, kind="ExternalInput")
    bias_d = nc.dram_tensor("bias", [1, D_GOAL], F32, kind="ExternalInput")
    out_d = nc.dram_tensor("out", [R, D_GOAL], F32, kind="ExternalOutput")

    with tile.TileContext(nc) as tc:
        with (
            tc.tile_pool(name="const", bufs=1) as constp,
            tc.tile_pool(name="scan", bufs=1) as scanp,
                        tc.tile_pool(name="xg", bufs=5) as xgp,
            tc.tile_pool(name="xg16", bufs=6) as xg16p,
            tc.tile_pool(name="xt", bufs=8) as xtp,
            tc.tile_pool(name="ysb", bufs=3) as yp,
            tc.tile_pool(name="pscan", bufs=1, space="PSUM") as pscan,
            tc.tile_pool(name="ptr", bufs=3, space="PSUM") as ptr,
            tc.tile_pool(name="pmm", bufs=4, space="PSUM") as pmm,
        ):
            # ---- constants ----
            ident = constp.tile([P, P], F32)
            make_identity(nc, ident[:])
            use_bf16 = gemm == "bf16"
            if use_bf16:
                ident16 = constp.tile([P, P], BF16)
                make_identity(nc, ident16[:])

            # ---- forward-fill index computation ----
            # Scan layout: tile [128, 128] viewed [pi, B, q]: element (pi,B,q)
            # holds row r = 2048*B + 16*pi + q.
            msk = scanp.tile([P, P], U8)
            nc.sync.dma_start(
                out=msk[:].rearrange("p (B q) -> p B q", q=16),
                in_=msk_d[:]
                .rearrange("a b -> (a b)")
                .rearrange("(B pi q) -> pi B q", B=NB, pi=P, q=16),
            )
            riota = scanp.tile([P, P], I32)  # r+1 values
            nc.gpsimd.iota(
                riota[:].rearrange("p (B q) -> p B q", q=16),
                pattern=[[P * 16, NB], [1, 16]],
                base=1,
                channel_multiplier=16,
            )
            riota_f = scanp.tile([P, P], F32)
            nc.vector.tensor_copy(out=riota_f[:], in_=riota[:])

            # u[r] = (r+1) if mask[r] else -1
            u = scanp.tile([P, P], F32)
            nc.vector.memset(u[:], -1.0)
            nc.vector.copy_predicated(out=u[:], mask=msk[:], data=riota_f[:])

            # inclusive within-partition chain scan (free order = (B, q))
            chain = scanp.tile([P, P], F32)
            nc.vector.tensor_tensor_scan(
                out=chain[:],
                data0=u[:],
                data1=u[:],
                initial=-1.0,
                op0=mybir.AluOpType.max,
                op1=mybir.AluOpType.max,
            )
            chain3 = chain[:].rearrange("p (B q) -> p B q", q=16)

            # run-carry machinery: runlast[pi, B] -> exclusive carry CIN[pi, B]
            ps1 = pscan.tile([P, P], F32, tag="ps")
            nc.tensor.transpose(out=ps1[:NB, :], in_=chain3[:, :, 15], identity=ident[:])
            rlt = scanp.tile([NB, P], F32)
            nc.vector.tensor_copy(out=rlt[:], in_=ps1[:NB, :])
            rlti = scanp.tile([NB, P], F32)
            nc.vector.tensor_tensor_scan(
                out=rlti[:],
                data0=rlt[:],
                data1=rlt[:],
                initial=-1.0,
                op0=mybir.AluOpType.max,
                op1=mybir.AluOpType.max,
            )
            # exclusive along pi
            rlte = scanp.tile([NB, P], F32)
            nc.vector.memset(rlte[:], -1.0)
            nc.vector.tensor_copy(out=rlte[:, 1:], in_=rlti[:, : P - 1])
            # mini chain across B blocks
            ps2 = pscan.tile([P, P], F32, tag="ps")
            nc.tensor.transpose(
                out=ps2[:1, :NB], in_=rlti[:, P - 1 : P], identity=ident[:NB, :NB]
            )
            mrow = scanp.tile([1, NB], F32)
            nc.vector.tensor_copy(out=mrow[:], in_=ps2[:1, :NB])
            mrowi = scanp.tile([1, NB], F32)
            nc.vector.tensor_tensor_scan(
                out=mrowi[:],
                data0=mrow[:],
                data1=mrow[:],
                initial=-1.0,
                op0=mybir.AluOpType.max,
                op1=mybir.AluOpType.max,
            )
            mrowe = scanp.tile([1, NB], F32)
            nc.vector.memset(mrowe[:], -1.0)
            nc.vector.tensor_copy(out=mrowe[:, 1:], in_=mrowi[:, : NB - 1])
            ps3 = pscan.tile([P, P], F32, tag="ps")
            nc.tensor.transpose(
                out=ps3[:NB, :1], in_=mrowe[:], identity=ident[:1, :1]
            )
            mcol = scanp.tile([NB, 1], F32)
            nc.vector.tensor_copy(out=mcol[:], in_=ps3[:NB, :1])
            carry_t = scanp.tile([NB, P], F32)
            nc.vector.tensor_scalar(
                out=carry_t[:],
                in0=rlte[:],
                scalar1=mcol[:],
                scalar2=None,
                op0=mybir.AluOpType.max,
            )
            ps4 = pscan.tile([P, P], F32, tag="ps")
            nc.tensor.transpose(
                out=ps4[:, :NB], in_=carry_t[:], identity=ident[:NB, :NB]
            )
            cin = scanp.tile([P, NB], F32)
            nc.vector.tensor_copy(out=cin[:], in_=ps4[:, :NB])

            # exclusive index: ex = max(shift_q(chain), CIN); idx = max(ex, base)
            exq = scanp.tile([P, P], F32)
            exq3 = exq[:].rearrange("p (B q) -> p B q", q=16)
            nc.vector.memset(exq[:], -1.0)
            nc.vector.tensor_copy(out=exq3[:, :, 1:], in_=chain3[:, :, : 15])
            exm = scanp.tile([P, P], F32)
            nc.vector.tensor_tensor(
                out=exm[:].rearrange("p (B q) -> p B q", q=16),
                in0=exq3,
                in1=cin[:, :, None].to_broadcast([P, NB, 16]),
                op=mybir.AluOpType.max,
            )
            # base[r] = 4096*(r//4096); with r = 2048B+16pi+q -> depends on B//2
            base_i = scanp.tile([P, NB // 2], I32)
            nc.gpsimd.iota(
                base_i[:], pattern=[[S, NB // 2]], base=0, channel_multiplier=0
            )
            base_f = scanp.tile([P, NB // 2], F32)
            nc.vector.tensor_copy(out=base_f[:], in_=base_i[:])
            idxf = scanp.tile([P, P], F32)
            nc.vector.tensor_tensor(
                out=idxf[:].rearrange("p (bh bl q) -> p bh bl q", bh=NB // 2, bl=2),
                in0=exm[:].rearrange("p (bh bl q) -> p bh bl q", bh=NB // 2, bl=2),
                in1=base_f[:, :, None, None].to_broadcast([P, NB // 2, 2, 16]),
                op=mybir.AluOpType.max,
            )

            # wrap into dma_gather's int16 index layout:
            # idxw[q + 16*rep, g*32 + s] = idx(512g + 16s + q)
            idxf3 = idxf[:].rearrange("p (B q) -> p B q", q=16)
            idxw = constp.tile([P, N_G * 32], I16)
            for Bb in range(NB):
                rep = scanp.tile([P, P], F32, tag="rep")
                nc.vector.tensor_copy(
                    out=rep[:].rearrange("p (B q) -> p B q", q=16),
                    in_=idxf3[:, Bb : Bb + 1, :].to_broadcast([P, NB, 16]),
                )
                psb = pscan.tile([P, P], F32, tag="ps")
                nc.tensor.transpose(out=psb[:], in_=rep[:], identity=ident[:])
                nc.vector.tensor_copy(out=idxw[:, ts(Bb, P)], in_=psb[:])


            bias_ld = constp.tile([1, D_GOAL], F32)
            nc.sync.dma_start(out=bias_ld[:], in_=bias_d[:])
            # broadcast bias to all 128 partitions via a K=1 ones-matmul
            ones_row = constp.tile([1, P], F32)
            nc.vector.memset(ones_row[:], 1.0)
            psbias = pmm.tile([P, D_GOAL], F32, tag="mm")
            nc.tensor.matmul(
                out=psbias[:], lhsT=ones_row[:], rhs=bias_ld[:], start=True, stop=True
            )
            bias_bc = constp.tile([P, D_GOAL], F32)
            nc.vector.tensor_copy(out=bias_bc[:], in_=psbias[:])

            # ---- W^T: load W [512,1024] then 32 PE transposes into wt ----
            wload = xgp.tile([P, 4, D_IN], F32, tag="xg")
            nc.sync.dma_start(
                out=wload[:],
                in_=w_d[:].rearrange("(i p) d -> p i d", p=P),
            )
            wt = constp.tile([P, K_TILES * D_GOAL], matmul_dtype)  # k-tile k
            for k in range(K_TILES):
                psw = ptr.tile([P, D_GOAL], F32, tag="tr")
                for i in range(4):
                    nc.tensor.transpose(
                        out=psw[:, ts(i, P)],
                        in_=wload[:, i, ts(k, P)],
                        identity=ident[:],
                    )
                nc.vector.tensor_copy(out=wt[:, ts(k, D_GOAL)], in_=psw[:])

            # ---- main loop: gather -> transpose -> matmul -> bias -> store ----
            for g in range(N_G):
                xg = xgp.tile([P, CHUNKS_PER_G, D_IN], F32)
                nc.gpsimd.dma_gather(
                    xg[:],
                    x_d[:],
                    idxw[:, ts(g, 32)],
                    num_idxs=N_GATHER,
                    num_idxs_reg=N_GATHER,
                    elem_size=D_IN,
                    queue_num=g % NQ,
                )
                if use_bf16:
                    xs = xg16p.tile([P, CHUNKS_PER_G, D_IN], BF16)
                    for j in range(CHUNKS_PER_G):
                        if j % 2 == 0:
                            nc.vector.tensor_copy(out=xs[:, j], in_=xg[:, j])
                        else:
                            nc.scalar.copy(out=xs[:, j], in_=xg[:, j])
                else:
                    xs = xg
                for j in range(CHUNKS_PER_G):
                    if use_bf16:
                        psT = ptr.tile([P, D_IN], BF16, tag="tr")
                        for k in range(K_TILES):
                            nc.tensor.transpose(
                                out=psT[:, ts(k, P)],
                                in_=xs[:, j, ts(k, P)],
                                identity=ident16[:],
                            )
                        xt = xtp.tile([P, D_IN], BF16)
                        if (g * CHUNKS_PER_G + j) % 2 == 0:
                            nc.vector.tensor_copy(out=xt[:], in_=psT[:])
                        else:
                            nc.scalar.copy(out=xt[:], in_=psT[:])
                    else:
                        psA = ptr.tile([P, 4 * P], F32, tag="tr")
                        psB = ptr.tile([P, 4 * P], F32, tag="tr")
                        for k in range(K_TILES):
                            dst = psA if k < 4 else psB
                            nc.tensor.transpose(
                                out=dst[:, ts(k % 4, P)],
                                in_=xs[:, j, ts(k, P)],
                                identity=ident[:],
                            )
                        xt = xtp.tile([P, D_IN], matmul_dtype)
                        nc.vector.tensor_copy(out=xt[:, : 4 * P], in_=psA[:])
                        nc.scalar.copy(out=xt[:, 4 * P :], in_=psB[:])
                    psy = pmm.tile([P, D_GOAL], F32, tag="mm")
                    for k in range(K_TILES):
                        nc.tensor.matmul(
                            out=psy[:],
                            lhsT=xt[:, ts(k, P)],
                            rhs=wt[:, ts(k, D_GOAL)],
                            start=(k == 0),
                            stop=(k == K_TILES - 1),
                        )
                    if j == 0:
                        ysb = yp.tile([P, CHUNKS_PER_G, D_GOAL], F32)
                    nc.vector.tensor_tensor(
                        out=ysb[:, j],
                        in0=psy[:],
                        in1=bias_bc[:],
                        op=mybir.AluOpType.add,
                    )
                    if j == CHUNKS_PER_G - 1:
                        r0 = g * N_GATHER
                        nc.sync.dma_start(
                            out=out_d[r0 : r0 + N_GATHER, :].rearrange(
                                "(j p) g -> p j g", p=P
                            ),
                            in_=ysb[:],
                        )

    nc.compile()
    return nc


_CACHED = {}


def _get_program(**kw):
    key = tuple(sorted(kw.items()))
    if key not in _CACHED:
        _CACHED[key] = build_program(**kw)
    return _CACHED[key]


def make_in_maps(x, critic_mask, W, b):
    x = np.ascontiguousarray(np.asarray(x, dtype=np.float32))
    msk = np.asarray(critic_mask).astype(np.uint8)
    W = np.ascontiguousarray(np.asarray(W, dtype=np.float32))
    b = np.ascontiguousarray(np.asarray(b, dtype=np.float32)).reshape(1, D_GOAL)
    in_maps = []
    for c in range(N_CORES):
        sl = slice(c * B_PC, (c + 1) * B_PC)
        in_maps.append(
            {
                "x": x[sl].reshape(R, D_IN),
                "msk": msk[sl],
                "w": W,
                "bias": b,
            }
        )
    return in_maps


def kernel(x, critic_mask, W, b, _trace=False, **run_kw):
    nc = _get_program()
    in_maps = make_in_maps(x, critic_mask, W, b)
    res = run_bass_kernel_spmd(
        nc, in_maps, core_ids=list(range(N_CORES)), trace=_trace, **run_kw
    )
    out = np.stack([res.results[c]["out"] for c in range(N_CORES)])
    out = out.reshape(B_FULL, S, D_GOAL)
    if _trace:
        kernel.last_results = res
    return out


if __name__ == "__main__":
    rng = np.random.default_rng(0)
    x = rng.standard_normal((B_FULL, S, D_IN), dtype=np.float32)
    m = rng.integers(0, 2, size=(B_FULL, S)).astype(bool)
    W = rng.standard_normal((D_GOAL, D_IN), dtype=np.float32) / 32.0
    b = rng.standard_normal(D_GOAL).astype(np.float32) * 0.01
    out = kernel(x, m, W, b)
    print(out.shape, out.dtype)

